# revision 11
# baseline (speedup 1.0000x reference)
"""MultiHeadGeneralizedPooling Trainium2 kernel.

Data-parallel over batch: 32 batches -> 8 cores x 4 batches.
Per core, everything is computed in "feature-major" layout (feature dim on
SBUF partitions, sequence on the free axis):

  Hi^T (d, s)  = P_cat^T @ X^T          TensorE; k-tiles 0-3 in fp8e4m3 with
                                        perf_mode=DoubleRow (2 k-slabs/instr),
                                        k-tiles 4-5 in bf16. Weights pre-
                                        scaled x8 on host; copy-out applies
                                        scale 1/8 + P bias.
  A1^T (dh, s) = relu(W1^T @ Hi^T + b1) bf16, FD=1024 PSUM evacuation
  A2^T (d, s)  = W2^T @ A1^T            bf16, 3 k-tiles
  E            = exp(A2^T)              ScalarE; accum_out -> Z per partition
                                        (W2 bias dropped: softmax-invariant)
  u[d]         = sum_s E * Hi           DVE scalar_tensor_tensor accum,
                                        one FD=1024 op per s-half
  v            = u / Z + c              c = host-computed DC correction for
                                        the fp8 quantization of X and P:
                                        c = mean_s(Hi_exact - Hi_quant),
                                        exact because softmax weights sum to 1

Host side pre-transposes/casts X to X^T (fp8 k-tiles 0-3, bf16 4-5) and
packs the (tiny) weights into lhsT layouts, so the device does no
transposes of the big tensor.
"""

import numpy as np
from contextlib import ExitStack

B, S, T = 32, 2048, 768
NH, DH, DHID = 8, 96, 384
NCORES = 8
BPC = B // NCORES  # batches per core
KT = T // 128      # 6 contraction tiles
KF8 = 4            # k-tiles 0..3 in fp8 (DoubleRow), 4..5 in bf16
KBF = KT - KF8
DT = (NH * DH) // 128  # 6 d-tiles of the packed head dim
SC = 4             # s-chunks per batch
SCW = S // SC      # 512
SH = 2             # s-halves per batch (MLP unit granularity)
SHW = S // SH      # 1024
KC = DHID // 128   # 3
PSCALE = 8.0       # host pre-scale of P (both fp8 and bf16 parts)
import os
N_WARM = int(os.environ.get("K_NWARM", "32"))  # PE clock-gate warmup matmuls
FAST_WARM = os.environ.get("K_FASTWARM", "1") == "1"
LEAD = int(os.environ.get("K_LEAD", "8"))
MRAT = int(os.environ.get("K_MRAT", "7"))   # pace M when m_i*MRATD <= p_i*MRAT
MRATD = int(os.environ.get("K_MRATD", "10"))
SPLITP = int(os.environ.get("K_SPLITP", "72"))  # p_i >= SPLITP: dual-engine a1 evac

_NC_CACHE = {}


def _segs():
    """Per projection d-tile: (psum_row, head, head_row, nrows) segments
    mapping packed d rows (128*dt + p) onto per-head (h, q<96) layout."""
    segs = []
    for dt in range(DT):
        cur, d0, d1 = [], 128 * dt, 128 * (dt + 1)
        d = d0
        while d < d1:
            h, q = d // DH, d % DH
            n = min(d1 - d, DH - q)
            cur.append((d - d0, h, q, n))
            d += n
        segs.append(cur)
    return segs


def _build_nc():
    import concourse.bacc as bacc
    import concourse.tile as tile
    from concourse import mybir

    f32 = mybir.dt.float32
    bf16 = mybir.dt.bfloat16
    f8 = mybir.dt.float8e4
    AF = mybir.ActivationFunctionType
    OP = mybir.AluOpType
    AX = mybir.AxisListType
    DR = mybir.MatmulPerfMode.DoubleRow

    nc = bacc.Bacc()
    # partition-major xt layout: [b, p, kt, s] matches the SBUF tile dim
    # order so quarter/full loads are single descriptors
    xt8 = nc.declare_dram_parameter("xt8", [BPC, 128, KF8, S], f8, isOutput=False)
    xtb = nc.declare_dram_parameter("xtb", [BPC, 128, KBF, S], bf16, isOutput=False)
    p8 = nc.declare_dram_parameter("p8", [128, KF8, NH * DH], f8, isOutput=False)
    pb16 = nc.declare_dram_parameter("pb16", [128, KBF, NH * DH], bf16, isOutput=False)
    w1 = nc.declare_dram_parameter("w1", [DH, NH, DHID], bf16, isOutput=False)
    b1l = nc.declare_dram_parameter("b1l", [128, NH, KC], f32, isOutput=False)
    w2 = nc.declare_dram_parameter("w2", [128, NH, KC, DH], bf16, isOutput=False)
    pb = nc.declare_dram_parameter("pb", [128, DT], f32, isOutput=False)
    cvec = nc.declare_dram_parameter("cvec", [DH, BPC * NH], f32, isOutput=False)
    ident = nc.declare_dram_parameter("ident", [DH, DH], f32, isOutput=False)
    out = nc.declare_dram_parameter("out", [BPC * NH, DH], f32, isOutput=True)

    segs = _segs()

    with tile.TileContext(nc) as tc:
        with ExitStack() as ctx:
            singles = ctx.enter_context(tc.tile_pool(name="singles", bufs=1))
            xt_pool = ctx.enter_context(tc.tile_pool(name="xtp", bufs=2))
            flat_pool = ctx.enter_context(tc.tile_pool(name="flat", bufs=7))
            a1sb_pool = ctx.enter_context(tc.tile_pool(name="a1sb", bufs=3))
            e_pool = ctx.enter_context(tc.tile_pool(name="ep", bufs=3))
            stt_pool = ctx.enter_context(tc.tile_pool(name="sttp", bufs=3))
            small_pool = ctx.enter_context(tc.tile_pool(name="small", bufs=4))
            pp_pool = ctx.enter_context(tc.tile_pool(name="pp", bufs=2, space="PSUM"))
            a1p_pool = ctx.enter_context(tc.tile_pool(name="a1p", bufs=2, space="PSUM"))
            a2p_pool = ctx.enter_context(tc.tile_pool(name="a2p", bufs=2, space="PSUM"))

            # PE warmup: dummy matmuls while DMAs stream in. With FAST_WARM
            # the warm tiles are never written (values irrelevant, psum is
            # never read), so the PE starts the moment its queue opens.
            warm_sb = singles.tile([128, 128], bf16)
            warm_sb2 = singles.tile([128, 128], bf16)
            if not FAST_WARM:
                nc.vector.memset(warm_sb, 0.0)
                nc.vector.memset(warm_sb2, 0.0)
            for i in range(N_WARM):
                wp = pp_pool.tile([128, SCW], f32, tag="pp")
                nc.tensor.matmul(
                    wp[:, 0:128], warm_sb, warm_sb2, start=True, stop=True
                )
            if FAST_WARM:
                # Written only after all warm reads (WAR, not RAW): satisfies
                # the tile allocator without gating the PE on another engine.
                nc.gpsimd.memset(warm_sb, 0.0)
                nc.gpsimd.memset(warm_sb2, 0.0)

            # Projection inputs first (needed immediately). One descriptor per
            # tensor/quarter (the Sync engine's ~0.65us per-descriptor issue
            # cost dominates early latency, not wire time). b0's xt arrives
            # quarter-granular so the first P-unit is ready after ~1.3MB.
            p8_sb = singles.tile([128, KF8, NH * DH], f8)
            pb16_sb = singles.tile([128, KBF, NH * DH], bf16)
            xt8_t0 = xt_pool.tile([128, KF8, S], f8, tag="xt8")
            xtb_t0 = xt_pool.tile([128, KBF, S], bf16, tag="xtb")
            nc.sync.dma_start(out=p8_sb, in_=p8[:])
            nc.sync.dma_start(out=xt8_t0[:, :, 0:SCW], in_=xt8[0, :, :, 0:SCW])
            nc.sync.dma_start(out=pb16_sb, in_=pb16[:])
            nc.sync.dma_start(out=xtb_t0[:, :, 0:SCW], in_=xtb[0, :, :, 0:SCW])
            # tiny constants next: the first proj copies need pb and the
            # first MLP units need w1/b1l well before the later quarters
            pb_sb = singles.tile([128, DT], f32)
            nc.sync.dma_start(out=pb_sb, in_=pb[:])
            b1l_sb = singles.tile([128, NH, KC], f32)
            nc.sync.dma_start(out=b1l_sb, in_=b1l[:])
            nc.sync.dma_start(out=xt8_t0[:, :, SCW:2 * SCW], in_=xt8[0, :, :, SCW:2 * SCW])
            nc.sync.dma_start(out=xtb_t0[:, :, SCW:2 * SCW], in_=xtb[0, :, :, SCW:2 * SCW])
            w1_sb = singles.tile([DH, NH, DHID], bf16)
            nc.sync.dma_start(out=w1_sb, in_=w1[:])
            nc.sync.dma_start(out=xt8_t0[:, :, 2 * SCW:S], in_=xt8[0, :, :, 2 * SCW:S])
            nc.sync.dma_start(out=xtb_t0[:, :, 2 * SCW:S], in_=xtb[0, :, :, 2 * SCW:S])
            w2_sb = singles.tile([128, NH, KC, DH], bf16)
            nc.sync.dma_start(out=w2_sb, in_=w2[:])
            id_sb = singles.tile([DH, DH], f32)
            nc.sync.dma_start(out=id_sb, in_=ident[:])
            cv_sb = singles.tile([DH, BPC * NH], f32)
            nc.sync.dma_start(out=cv_sb, in_=cvec[:])
            v_sb = singles.tile([DH, BPC * NH, SH], f32)
            zr_sb = singles.tile([DH, BPC * NH], f32)

            # Hi^T per-head buffers (bf16); two manual buffers so batch b+1's
            # projection overlaps batch b's MLP.
            hh = []
            for i in range(2):
                t = singles.tile([DH, NH, S], bf16, tag=f"hh{i}", name=f"hh{i}")
                hh.append(t)

            # greedy ACT/DVE balance for PSUM evacuations (exp forced ACT,
            # stt forced DVE; both pre-loaded into the counters as they go)
            eng_ns = {"act": 0.0, "dve": 0.0}

            def evac_engine(act_ns, dve_ns):
                if eng_ns["act"] + act_ns <= eng_ns["dve"] + dve_ns:
                    eng_ns["act"] += act_ns
                    return "act"
                eng_ns["dve"] += dve_ns
                return "dve"

            def proj_quarter(b, xt8_t, xtb_t, dt, sc, flat_t):
                ps = pp_pool.tile([128, SCW], f32, tag="pp")
                c0, c1 = SCW * sc, SCW * (sc + 1)
                for i in range(KF8 // 2):
                    nc.tensor.matmul(
                        ps,
                        p8_sb[:, 2 * i:2 * i + 2, 128 * dt:128 * (dt + 1)],
                        xt8_t[:, 2 * i:2 * i + 2, c0:c1],
                        start=(i == 0),
                        stop=False,
                        perf_mode=DR,
                    )
                for j in range(KBF):
                    nc.tensor.matmul(
                        ps,
                        pb16_sb[:, j, 128 * dt:128 * (dt + 1)],
                        xtb_t[:, j, c0:c1],
                        start=False,
                        stop=(j == KBF - 1),
                    )
                # copy-out applies the 1/PSCALE weight descale + P bias
                if evac_engine(590.0, 680.0) == "act":
                    nc.scalar.activation(
                        out=flat_t[:, c0:c1],
                        in_=ps,
                        func=AF.Identity,
                        bias=pb_sb[:, dt:dt + 1],
                        scale=1.0 / PSCALE,
                    )
                else:
                    nc.vector.tensor_scalar(
                        out=flat_t[:, c0:c1],
                        in0=ps,
                        scalar1=1.0 / PSCALE,
                        scalar2=pb_sb[:, dt:dt + 1],
                        op0=OP.mult,
                        op1=OP.add,
                    )

            def emit_remap_dtile(b, dt, scp, flat_t):
                hcur = hh[b % 2]
                s0, s1 = SHW * scp, SHW * (scp + 1)
                for (r0, h, q0, n) in segs[dt]:
                    nc.sync.dma_start(
                        out=hcur[q0:q0 + n, h, s0:s1],
                        in_=flat_t[r0:r0 + n, s0:s1],
                    )

            def mlp_half(b, h, sp, e_t, zp, split_evac=False):
                """One MLP s-half: W1+relu (FD=1024), W2+exp per s-quarter,
                one stt weighted-sum accumulation over the half. With
                split_evac (M-only tail) each relu evac runs as two FD=512
                ops on both engines so the a1p buffer recycles ~2x faster."""
                hcur = hh[b % 2]
                col = b * NH + h
                s0 = SHW * sp
                a1sb = a1sb_pool.tile([128, KC, SHW], bf16)
                for c in range(KC):
                    a1p = a1p_pool.tile([128, SHW], f32, tag="a1p")
                    for half in range(2):
                        nc.tensor.matmul(
                            a1p[:, SCW * half:SCW * (half + 1)],
                            w1_sb[:, h, 128 * c:128 * (c + 1)],
                            hcur[:, h, s0 + SCW * half:s0 + SCW * (half + 1)],
                            start=True,
                            stop=True,
                        )
                    if split_evac:
                        eng_ns["act"] += 590.0
                        eng_ns["dve"] += 680.0
                        act_half = c % 2  # alternate which engine gets half 0
                        nc.scalar.activation(
                            out=a1sb[:, c, SCW * act_half:SCW * (act_half + 1)],
                            in_=a1p[:, SCW * act_half:SCW * (act_half + 1)],
                            func=AF.Relu,
                            bias=b1l_sb[:, h, c:c + 1],
                        )
                        dve_half = 1 - act_half
                        nc.vector.tensor_scalar(
                            out=a1sb[:, c, SCW * dve_half:SCW * (dve_half + 1)],
                            in0=a1p[:, SCW * dve_half:SCW * (dve_half + 1)],
                            scalar1=b1l_sb[:, h, c:c + 1], scalar2=0.0,
                            op0=OP.add, op1=OP.max,
                        )
                    elif evac_engine(1100.0, 1280.0) == "act":
                        nc.scalar.activation(
                            out=a1sb[:, c, :], in_=a1p, func=AF.Relu,
                            bias=b1l_sb[:, h, c:c + 1],
                        )
                    else:
                        nc.vector.tensor_scalar(
                            out=a1sb[:, c, :], in0=a1p,
                            scalar1=b1l_sb[:, h, c:c + 1], scalar2=0.0,
                            op0=OP.add, op1=OP.max,
                        )
                for half in range(2):
                    a2p = a2p_pool.tile([DH, SCW], f32, tag="a2p")
                    for kc in range(KC):
                        nc.tensor.matmul(
                            a2p,
                            w2_sb[:, h, kc, :],
                            a1sb[:, kc, SCW * half:SCW * (half + 1)],
                            start=(kc == 0),
                            stop=(kc == KC - 1),
                        )
                    sc = 2 * sp + half
                    eng_ns["act"] += 960.0
                    nc.scalar.activation(
                        out=e_t[:, SCW * half:SCW * (half + 1)],
                        in_=a2p,
                        func=AF.Exp,
                        accum_out=zp[:, sc:sc + 1],
                    )
                # weighted-sum accumulation over the whole half (FD=1024)
                stt_t = stt_pool.tile([DH, SHW], bf16)
                eng_ns["dve"] += 594.0
                nc.vector.scalar_tensor_tensor(
                    out=stt_t,
                    in0=e_t,
                    scalar=1.0,
                    in1=hcur[:, h, s0:s0 + SHW],
                    op0=OP.mult,
                    op1=OP.mult,
                    accum_out=v_sb[:, col, sp:sp + 1],
                )

            def mlp_finish(b, h):
                col = b * NH + h
                z1 = small_pool.tile([DH, 1], f32, tag="z1")
                nc.vector.tensor_reduce(
                    out=z1, in_=small_state[(b, h)], axis=AX.X, op=OP.add
                )
                nc.vector.reciprocal(zr_sb[:, col:col + 1], z1)

            # --- ratio-paced scheduler ---
            # P units: (b, dt, sc) projection quarters; M units: (b, h, sp)
            # MLP halves. M gated on its head's remaps.
            dts_of = {}
            for h in range(NH):
                dts_of[h] = sorted({(DH * h) // 128, (DH * h + DH - 1) // 128})
            # b0 runs in sc-pair-major order so it only needs the first-half
            # xt columns for its first 12 P-units; later batches are fully
            # prefetched, so plain dt-major order is fine there.
            P_units = [(0, dt, 2 * sp + s) for sp in range(2)
                       for dt in range(DT) for s in range(2)]
            P_units += [(b, dt, sc) for b in range(1, BPC) for dt in range(DT)
                        for sc in range(SC)]
            M_units = [(0, h, sp) for sp in range(2) for h in range(NH)]
            M_units += [(b, h, sp) for b in range(1, BPC) for h in range(NH)
                        for sp in range(SH)]
            flat_state = {}
            small_state = {}
            remapped = set()
            xt_tiles = {0: (xt8_t0, xtb_t0)}
            p_i = m_i = 0

            def emit_P():
                nonlocal p_i
                b, dt, sc = P_units[p_i]
                # prefetch b+1: two single-descriptor whole-batch DMAs,
                # spread across dt 2 and 4 to avoid wire bursts
                if dt in (2, 4) and sc == 0 and b + 1 < BPC:
                    if dt == 2:
                        xt_tiles[b + 1] = (
                            xt_pool.tile([128, KF8, S], f8, tag="xt8", name="nx8"),
                            xt_pool.tile([128, KBF, S], bf16, tag="xtb", name="nxb"),
                        )
                    nx8, nxb = xt_tiles[b + 1]
                    if dt == 2:
                        nc.sync.dma_start(out=nx8, in_=xt8[b + 1])
                    else:
                        nc.sync.dma_start(out=nxb, in_=xtb[b + 1])
                if sc == 0:
                    flat_state[(b, dt)] = flat_pool.tile([128, S], bf16, tag="flat", name="flat_t")
                x8t, xbt = xt_tiles[b]
                proj_quarter(b, x8t, xbt, dt, sc, flat_state[(b, dt)])
                if sc % 2 == 1:
                    emit_remap_dtile(b, dt, sc // 2, flat_state[(b, dt)])
                    remapped.add((b, dt, sc // 2))
                if sc == SC - 1:
                    flat_state.pop((b, dt))
                p_i += 1

            def emit_M():
                nonlocal m_i
                b, h, sp = M_units[m_i]
                if (b, h) not in small_state:
                    small_state[(b, h)] = small_pool.tile(
                        [DH, SC], f32, tag="zp", name="zp", bufs=12
                    )
                e_t = e_pool.tile([DH, SHW], bf16, tag="e_t", name="e_t")
                zp = small_state[(b, h)]
                mlp_half(b, h, sp, e_t, zp, split_evac=(p_i >= SPLITP))
                if sp == SH - 1:
                    mlp_finish(b, h)
                    del small_state[(b, h)]
                m_i += 1

            def m_ready():
                if m_i >= len(M_units):
                    return False
                b, h, sp = M_units[m_i]
                return all((b, dt, sp) in remapped for dt in dts_of[h])

            while p_i < len(P_units) or m_i < len(M_units):
                lead = LEAD if p_i < len(P_units) - 16 else 0
                # pace M slightly faster than the 2:3 steady-state ratio so
                # the engine-bound M backlog is small when P runs out
                want_m = m_i * MRATD <= (p_i - lead) * MRAT
                if p_i < len(P_units) and not (want_m and m_ready()):
                    emit_P()
                elif m_ready():
                    emit_M()
                elif p_i < len(P_units):
                    emit_P()
                else:
                    # only unready M left: emit in order anyway (deps safe)
                    emit_M()

            # v = (sum of per-half accums) / Z + c, then transpose (96, 32)
            # -> (32, 96) and store.
            vq_sb = singles.tile([DH, BPC * NH], f32)
            nc.vector.tensor_reduce(
                out=vq_sb, in_=v_sb, axis=AX.X, op=OP.add
            )
            vn_sb = singles.tile([DH, BPC * NH], f32)
            nc.vector.tensor_mul(vn_sb, vq_sb, zr_sb)
            vc_sb = singles.tile([DH, BPC * NH], f32)
            nc.vector.tensor_add(vc_sb, vn_sb, cv_sb)
            vout_p = a2p_pool.tile([BPC * NH, DH], f32, tag="a2p")
            nc.tensor.matmul(vout_p, vc_sb, id_sb, start=True, stop=True)
            out_sb = singles.tile([BPC * NH, DH], f32)
            nc.scalar.copy(out=out_sb, in_=vout_p)
            nc.sync.dma_start(out=out[:], in_=out_sb)
    nc.compile()
    return nc


def get_nc():
    if "nc" not in _NC_CACHE:
        _NC_CACHE["nc"] = _build_nc()
    return _NC_CACHE["nc"]


def make_in_maps(token_embeddings, P_w, P_b, W1_w, W1_b, W2_w, W2_b):
    import ml_dtypes

    bf16 = ml_dtypes.bfloat16
    f8 = ml_dtypes.float8_e4m3
    TF8 = KF8 * 128  # features 0..511 in fp8

    X = np.asarray(token_embeddings, dtype=np.float32)
    # X^T per batch: (B, T, S) -> partition-major tiles [b, p, kt, s]
    XT = np.ascontiguousarray(X.transpose(0, 2, 1))
    XT8 = np.ascontiguousarray(
        XT[:, :TF8].astype(f8).reshape(B, KF8, 128, S).transpose(0, 2, 1, 3)
    )
    XTB = np.ascontiguousarray(
        XT[:, TF8:].astype(bf16).reshape(B, KBF, 128, S).transpose(0, 2, 1, 3)
    )

    P_cat = np.transpose(np.asarray(P_w, np.float32), (1, 0, 2)).reshape(T, NH * DH)
    P_s = P_cat * PSCALE
    p8_np = np.ascontiguousarray(
        P_s[:TF8].reshape(KF8, 128, NH * DH).transpose(1, 0, 2)
    ).astype(f8)
    pb16_np = np.ascontiguousarray(
        P_s[TF8:].reshape(KBF, 128, NH * DH).transpose(1, 0, 2)
    ).astype(bf16)

    # DC correction: c = mean_s(Hi_exact - Hi_quant), per (b, h*d).
    # Effective quantized P as used on device (scale folded back out):
    Pq = np.concatenate([
        p8_np.transpose(1, 0, 2).reshape(TF8, NH * DH).astype(np.float32),
        pb16_np.transpose(1, 0, 2).reshape(KBF * 128, NH * DH).astype(np.float32),
    ], axis=0) / PSCALE
    mu = X.mean(axis=1, dtype=np.float64)  # (B, T) exact means
    # means of the quantized X, reordered from [b, p, kt] to kt-major T index
    muq8 = XT8.astype(np.float32).mean(axis=3, dtype=np.float64)  # (B, 128, KF8)
    muqb = XTB.astype(np.float32).mean(axis=3, dtype=np.float64)  # (B, 128, KBF)
    muq = np.concatenate([
        muq8.transpose(0, 2, 1).reshape(B, TF8),
        muqb.transpose(0, 2, 1).reshape(B, KBF * 128),
    ], axis=1)  # (B, T)
    cfull = (mu @ P_cat.astype(np.float64)
             - muq @ Pq.astype(np.float64)).astype(np.float32)  # (B, NH*DH)

    w1 = np.ascontiguousarray(
        np.asarray(W1_w, np.float32).transpose(1, 0, 2)
    ).astype(bf16)
    b1l = np.ascontiguousarray(
        np.asarray(W1_b, np.float32).reshape(NH, KC, 128).transpose(2, 0, 1)
    ).astype(np.float32)

    w2 = np.ascontiguousarray(
        np.asarray(W2_w, np.float32).reshape(NH, KC, 128, DH).transpose(2, 0, 1, 3)
    ).astype(bf16)

    pb = np.ascontiguousarray(
        np.asarray(P_b, np.float32).reshape(NH * DH).reshape(KT, 128).T
    ).astype(np.float32)
    ident = np.eye(DH, dtype=np.float32)

    in_maps = []
    for c in range(NCORES):
        # cvec[d, b*NH+h] matches v_sb column layout
        cv = np.ascontiguousarray(
            cfull[c * BPC:(c + 1) * BPC].reshape(BPC * NH, DH).T
        ).astype(np.float32)
        in_maps.append({
            "xt8": np.ascontiguousarray(XT8[c * BPC:(c + 1) * BPC]),
            "xtb": np.ascontiguousarray(XTB[c * BPC:(c + 1) * BPC]),
            "p8": p8_np,
            "pb16": pb16_np,
            "w1": w1,
            "b1l": b1l,
            "w2": w2,
            "pb": pb,
            "cvec": cv,
            "ident": ident,
        })
    return in_maps


def _reference_host(token_embeddings, attention_mask, P_w, P_b, W1_w, W1_b, W2_w, W2_b):
    """Exact numpy fallback (only used if the mask is not all-ones)."""
    X = np.asarray(token_embeddings, np.float64)
    Hi = np.einsum("bst,htd->bhsd", X, np.asarray(P_w, np.float64))
    Hi += np.asarray(P_b, np.float64)[None, :, None, :]
    A = np.einsum("bhsd,hde->bhse", Hi, np.asarray(W1_w, np.float64))
    A += np.asarray(W1_b, np.float64)[None, :, None, :]
    A = np.maximum(A, 0.0)
    A = np.einsum("bhse,hed->bhsd", A, np.asarray(W2_w, np.float64))
    A += np.asarray(W2_b, np.float64)[None, :, None, :]
    with np.errstate(divide="ignore"):
        logm = np.log(np.asarray(attention_mask, np.float64))[:, None, :, None]
    A = A + logm
    A = A - A.max(axis=2, keepdims=True)
    E = np.exp(A)
    A = E / E.sum(axis=2, keepdims=True)
    v = (Hi * A).sum(axis=2)
    return v.reshape(v.shape[0], NH * DH).astype(np.float32)


def kernel(**inputs):
    mask = np.asarray(inputs["attention_mask"], np.float32)
    if not np.all(mask == 1.0):
        return _reference_host(**inputs)

    from concourse.bass_utils import run_bass_kernel_spmd

    nc = get_nc()
    in_maps = make_in_maps(
        inputs["token_embeddings"], inputs["P_w"], inputs["P_b"],
        inputs["W1_w"], inputs["W1_b"], inputs["W2_w"], inputs["W2_b"],
    )
    res = run_bass_kernel_spmd(nc, in_maps, core_ids=list(range(NCORES)))
    outs = [
        np.asarray(r["out"], np.float32).reshape(BPC, NH * DH)
        for r in res.results
    ]
    return np.concatenate(outs, axis=0)


# revision 13
# speedup vs baseline: 1.1288x; 1.1288x over previous
"""MultiHeadGeneralizedPooling Trainium2 kernel.

Data-parallel over batch: 32 batches -> 8 cores x 4 batches.
Per core, everything is computed in "feature-major" layout (feature dim on
SBUF partitions, sequence on the free axis):

  Hi^T (d, s)  = P_cat^T @ X^T          TensorE; k-tiles 0-3 in fp8e4m3 with
                                        perf_mode=DoubleRow (2 k-slabs/instr),
                                        k-tiles 4-5 in bf16. Weights pre-
                                        scaled x8 on host; copy-out applies
                                        scale 1/8 + P bias.
  A1^T (dh, s) = relu(W1^T @ Hi^T + b1) bf16, FD=1024 PSUM evacuation
  A2^T (d, s)  = W2^T @ A1^T            bf16, 3 k-tiles
  E            = exp(A2^T)              ScalarE; accum_out -> Z per partition
                                        (W2 bias dropped: softmax-invariant)
  u[d]         = sum_s E * Hi           DVE scalar_tensor_tensor accum,
                                        one FD=1024 op per s-half
  v            = u / Z + c              c = host-computed DC correction for
                                        the fp8 quantization of X and P:
                                        c = mean_s(Hi_exact - Hi_quant),
                                        exact because softmax weights sum to 1

Host side pre-transposes/casts X to X^T (fp8 k-tiles 0-3, bf16 4-5) and
packs the (tiny) weights into lhsT layouts, so the device does no
transposes of the big tensor.
"""

import numpy as np
from contextlib import ExitStack

B, S, T = 32, 2048, 768
NH, DH, DHID = 8, 96, 384
NCORES = 8
BPC = B // NCORES  # batches per core
KT = T // 128      # 6 contraction tiles
KF8 = 4            # k-tiles 0..3 in fp8 (DoubleRow), 4..5 in bf16
KBF = KT - KF8
DT = (NH * DH) // 128  # 6 d-tiles of the packed head dim
SC = 4             # s-chunks per batch
SCW = S // SC      # 512
SH = 2             # s-halves per batch (MLP unit granularity)
SHW = S // SH      # 1024
KC = DHID // 128   # 3
PSCALE = 8.0       # host pre-scale of P (both fp8 and bf16 parts)
import os
N_WARM = int(os.environ.get("K_NWARM", "32"))  # PE clock-gate warmup matmuls
FAST_WARM = os.environ.get("K_FASTWARM", "1") == "1"
LEAD = int(os.environ.get("K_LEAD", "8"))
MRAT = int(os.environ.get("K_MRAT", "7"))   # pace M when m_i*MRATD <= p_i*MRAT
MRATD = int(os.environ.get("K_MRATD", "10"))
SPLITP = int(os.environ.get("K_SPLITP", "72"))  # p_i >= SPLITP: dual-engine a1 evac

_NC_CACHE = {}


def _segs():
    """Per projection d-tile: (psum_row, head, head_row, nrows) segments
    mapping packed d rows (128*dt + p) onto per-head (h, q<96) layout."""
    segs = []
    for dt in range(DT):
        cur, d0, d1 = [], 128 * dt, 128 * (dt + 1)
        d = d0
        while d < d1:
            h, q = d // DH, d % DH
            n = min(d1 - d, DH - q)
            cur.append((d - d0, h, q, n))
            d += n
        segs.append(cur)
    return segs


def _build_nc():
    import concourse.bacc as bacc
    import concourse.tile as tile
    from concourse import mybir

    f32 = mybir.dt.float32
    bf16 = mybir.dt.bfloat16
    f8 = mybir.dt.float8e4
    AF = mybir.ActivationFunctionType
    OP = mybir.AluOpType
    AX = mybir.AxisListType
    DR = mybir.MatmulPerfMode.DoubleRow

    nc = bacc.Bacc()
    # partition-major xt layout: [b, p, kt, s] matches the SBUF tile dim
    # order so quarter/full loads are single descriptors
    xt8 = nc.declare_dram_parameter("xt8", [BPC, 128, KF8, S], f8, isOutput=False)
    xtb = nc.declare_dram_parameter("xtb", [BPC, 128, KBF, S], bf16, isOutput=False)
    p8 = nc.declare_dram_parameter("p8", [128, KF8, NH * DH], f8, isOutput=False)
    pb16 = nc.declare_dram_parameter("pb16", [128, KBF, NH * DH], bf16, isOutput=False)
    w1 = nc.declare_dram_parameter("w1", [DH, NH, DHID], bf16, isOutput=False)
    b1l = nc.declare_dram_parameter("b1l", [128, NH, KC], f32, isOutput=False)
    w2 = nc.declare_dram_parameter("w2", [128, NH, KC, DH], bf16, isOutput=False)
    pb = nc.declare_dram_parameter("pb", [128, DT], f32, isOutput=False)
    cvec = nc.declare_dram_parameter("cvec", [DH, BPC * NH], f32, isOutput=False)
    ident = nc.declare_dram_parameter("ident", [DH, DH], f32, isOutput=False)
    out = nc.declare_dram_parameter("out", [BPC * NH, DH], f32, isOutput=True)

    segs = _segs()

    with tile.TileContext(nc) as tc:
        with ExitStack() as ctx:
            singles = ctx.enter_context(tc.tile_pool(name="singles", bufs=1))
            xt_pool = ctx.enter_context(tc.tile_pool(name="xtp", bufs=2))
            flat_pool = ctx.enter_context(tc.tile_pool(name="flat", bufs=7))
            a1sb_pool = ctx.enter_context(tc.tile_pool(name="a1sb", bufs=3))
            e_pool = ctx.enter_context(tc.tile_pool(name="ep", bufs=3))
            stt_pool = ctx.enter_context(tc.tile_pool(name="sttp", bufs=3))
            small_pool = ctx.enter_context(tc.tile_pool(name="small", bufs=4))
            pp_pool = ctx.enter_context(tc.tile_pool(name="pp", bufs=2, space="PSUM"))
            a1p_pool = ctx.enter_context(tc.tile_pool(name="a1p", bufs=2, space="PSUM"))
            a2p_pool = ctx.enter_context(tc.tile_pool(name="a2p", bufs=2, space="PSUM"))

            # PE warmup: dummy matmuls while DMAs stream in. With FAST_WARM
            # the warm tiles are never written (values irrelevant, psum is
            # never read), so the PE starts the moment its queue opens.
            warm_sb = singles.tile([128, 128], bf16)
            warm_sb2 = singles.tile([128, 128], bf16)
            if not FAST_WARM:
                nc.vector.memset(warm_sb, 0.0)
                nc.vector.memset(warm_sb2, 0.0)
            for i in range(N_WARM):
                wp = pp_pool.tile([128, SCW], f32, tag="pp")
                nc.tensor.matmul(
                    wp[:, 0:128], warm_sb, warm_sb2, start=True, stop=True
                )
            if FAST_WARM:
                # Written only after all warm reads (WAR, not RAW): satisfies
                # the tile allocator without gating the PE on another engine.
                nc.gpsimd.memset(warm_sb, 0.0)
                nc.gpsimd.memset(warm_sb2, 0.0)

            # Projection inputs first (needed immediately). One descriptor per
            # tensor/quarter (the Sync engine's ~0.65us per-descriptor issue
            # cost dominates early latency, not wire time). b0's xt arrives
            # quarter-granular so the first P-unit is ready after ~1.3MB.
            p8_sb = singles.tile([128, KF8, NH * DH], f8)
            pb16_sb = singles.tile([128, KBF, NH * DH], bf16)
            xt8_t0 = xt_pool.tile([128, KF8, S], f8, tag="xt8")
            xtb_t0 = xt_pool.tile([128, KBF, S], bf16, tag="xtb")
            nc.sync.dma_start(out=p8_sb, in_=p8[:])
            nc.sync.dma_start(out=xt8_t0[:, :, 0:SCW], in_=xt8[0, :, :, 0:SCW])
            nc.sync.dma_start(out=pb16_sb, in_=pb16[:])
            nc.sync.dma_start(out=xtb_t0[:, :, 0:SCW], in_=xtb[0, :, :, 0:SCW])
            # tiny constants next: the first proj copies need pb and the
            # first MLP units need w1/b1l well before the later quarters
            pb_sb = singles.tile([128, DT], f32)
            nc.sync.dma_start(out=pb_sb, in_=pb[:])
            b1l_sb = singles.tile([128, NH, KC], f32)
            nc.sync.dma_start(out=b1l_sb, in_=b1l[:])
            nc.sync.dma_start(out=xt8_t0[:, :, SCW:2 * SCW], in_=xt8[0, :, :, SCW:2 * SCW])
            nc.sync.dma_start(out=xtb_t0[:, :, SCW:2 * SCW], in_=xtb[0, :, :, SCW:2 * SCW])
            w1_sb = singles.tile([DH, NH, DHID], bf16)
            nc.sync.dma_start(out=w1_sb, in_=w1[:])
            nc.sync.dma_start(out=xt8_t0[:, :, 2 * SCW:S], in_=xt8[0, :, :, 2 * SCW:S])
            nc.sync.dma_start(out=xtb_t0[:, :, 2 * SCW:S], in_=xtb[0, :, :, 2 * SCW:S])
            w2_sb = singles.tile([128, NH, KC, DH], bf16)
            nc.sync.dma_start(out=w2_sb, in_=w2[:])
            id_sb = singles.tile([DH, DH], f32)
            nc.sync.dma_start(out=id_sb, in_=ident[:])
            cv_sb = singles.tile([DH, BPC * NH], f32)
            nc.sync.dma_start(out=cv_sb, in_=cvec[:])
            v_sb = singles.tile([DH, BPC * NH, SH], f32)
            zr_sb = singles.tile([DH, BPC * NH], f32)

            # Hi^T per-head buffers (bf16); two manual buffers so batch b+1's
            # projection overlaps batch b's MLP.
            hh = []
            for i in range(2):
                t = singles.tile([DH, NH, S], bf16, tag=f"hh{i}", name=f"hh{i}")
                hh.append(t)

            # greedy ACT/DVE balance for PSUM evacuations (exp forced ACT,
            # stt forced DVE; both pre-loaded into the counters as they go)
            eng_ns = {"act": 0.0, "dve": 0.0}

            def evac_engine(act_ns, dve_ns):
                if eng_ns["act"] + act_ns <= eng_ns["dve"] + dve_ns:
                    eng_ns["act"] += act_ns
                    return "act"
                eng_ns["dve"] += dve_ns
                return "dve"

            def proj_quarter(b, xt8_t, xtb_t, dt, sc, flat_t):
                ps = pp_pool.tile([128, SCW], f32, tag="pp")
                c0, c1 = SCW * sc, SCW * (sc + 1)
                for i in range(KF8 // 2):
                    nc.tensor.matmul(
                        ps,
                        p8_sb[:, 2 * i:2 * i + 2, 128 * dt:128 * (dt + 1)],
                        xt8_t[:, 2 * i:2 * i + 2, c0:c1],
                        start=(i == 0),
                        stop=False,
                        perf_mode=DR,
                    )
                for j in range(KBF):
                    nc.tensor.matmul(
                        ps,
                        pb16_sb[:, j, 128 * dt:128 * (dt + 1)],
                        xtb_t[:, j, c0:c1],
                        start=False,
                        stop=(j == KBF - 1),
                    )
                # copy-out applies the 1/PSCALE weight descale + P bias
                if evac_engine(590.0, 680.0) == "act":
                    nc.scalar.activation(
                        out=flat_t[:, c0:c1],
                        in_=ps,
                        func=AF.Identity,
                        bias=pb_sb[:, dt:dt + 1],
                        scale=1.0 / PSCALE,
                    )
                else:
                    nc.vector.tensor_scalar(
                        out=flat_t[:, c0:c1],
                        in0=ps,
                        scalar1=1.0 / PSCALE,
                        scalar2=pb_sb[:, dt:dt + 1],
                        op0=OP.mult,
                        op1=OP.add,
                    )

            def emit_remap_dtile(b, dt, scp, flat_t):
                hcur = hh[b % 2]
                s0, s1 = SHW * scp, SHW * (scp + 1)
                for (r0, h, q0, n) in segs[dt]:
                    nc.sync.dma_start(
                        out=hcur[q0:q0 + n, h, s0:s1],
                        in_=flat_t[r0:r0 + n, s0:s1],
                    )

            def mlp_half(b, h, sp, e_t, zp, split_evac=False):
                """One MLP s-half: W1+relu (FD=1024), W2+exp per s-quarter,
                one stt weighted-sum accumulation over the half. With
                split_evac (M-only tail) each relu evac runs as two FD=512
                ops on both engines so the a1p buffer recycles ~2x faster."""
                hcur = hh[b % 2]
                col = b * NH + h
                s0 = SHW * sp
                a1sb = a1sb_pool.tile([128, KC, SHW], bf16)
                for c in range(KC):
                    a1p = a1p_pool.tile([128, SHW], f32, tag="a1p")
                    for half in range(2):
                        nc.tensor.matmul(
                            a1p[:, SCW * half:SCW * (half + 1)],
                            w1_sb[:, h, 128 * c:128 * (c + 1)],
                            hcur[:, h, s0 + SCW * half:s0 + SCW * (half + 1)],
                            start=True,
                            stop=True,
                        )
                    if split_evac:
                        # two FD=512 evacs, greedy-balanced: halves the a1p
                        # recycle latency without overloading either engine
                        for eh in range(2):
                            if evac_engine(590.0, 680.0) == "act":
                                nc.scalar.activation(
                                    out=a1sb[:, c, SCW * eh:SCW * (eh + 1)],
                                    in_=a1p[:, SCW * eh:SCW * (eh + 1)],
                                    func=AF.Relu,
                                    bias=b1l_sb[:, h, c:c + 1],
                                )
                            else:
                                nc.vector.tensor_scalar(
                                    out=a1sb[:, c, SCW * eh:SCW * (eh + 1)],
                                    in0=a1p[:, SCW * eh:SCW * (eh + 1)],
                                    scalar1=b1l_sb[:, h, c:c + 1], scalar2=0.0,
                                    op0=OP.add, op1=OP.max,
                                )
                    elif evac_engine(1100.0, 1280.0) == "act":
                        nc.scalar.activation(
                            out=a1sb[:, c, :], in_=a1p, func=AF.Relu,
                            bias=b1l_sb[:, h, c:c + 1],
                        )
                    else:
                        nc.vector.tensor_scalar(
                            out=a1sb[:, c, :], in0=a1p,
                            scalar1=b1l_sb[:, h, c:c + 1], scalar2=0.0,
                            op0=OP.add, op1=OP.max,
                        )
                for half in range(2):
                    a2p = a2p_pool.tile([DH, SCW], f32, tag="a2p")
                    for kc in range(KC):
                        nc.tensor.matmul(
                            a2p,
                            w2_sb[:, h, kc, :],
                            a1sb[:, kc, SCW * half:SCW * (half + 1)],
                            start=(kc == 0),
                            stop=(kc == KC - 1),
                        )
                    sc = 2 * sp + half
                    eng_ns["act"] += 960.0
                    nc.scalar.activation(
                        out=e_t[:, SCW * half:SCW * (half + 1)],
                        in_=a2p,
                        func=AF.Exp,
                        accum_out=zp[:, sc:sc + 1],
                    )
                # weighted-sum accumulation over the whole half (FD=1024)
                stt_t = stt_pool.tile([DH, SHW], bf16)
                eng_ns["dve"] += 594.0
                nc.vector.scalar_tensor_tensor(
                    out=stt_t,
                    in0=e_t,
                    scalar=1.0,
                    in1=hcur[:, h, s0:s0 + SHW],
                    op0=OP.mult,
                    op1=OP.mult,
                    accum_out=v_sb[:, col, sp:sp + 1],
                )

            def mlp_finish(b, h):
                col = b * NH + h
                z1 = small_pool.tile([DH, 1], f32, tag="z1")
                nc.vector.tensor_reduce(
                    out=z1, in_=small_state[(b, h)], axis=AX.X, op=OP.add
                )
                nc.vector.reciprocal(zr_sb[:, col:col + 1], z1)

            # --- ratio-paced scheduler ---
            # P units: (b, dt, sc) projection quarters; M units: (b, h, sp)
            # MLP halves. M gated on its head's remaps.
            dts_of = {}
            for h in range(NH):
                dts_of[h] = sorted({(DH * h) // 128, (DH * h + DH - 1) // 128})
            # b0 runs in sc-pair-major order so it only needs the first-half
            # xt columns for its first 12 P-units; later batches are fully
            # prefetched, so plain dt-major order is fine there.
            P_units = [(0, dt, 2 * sp + s) for sp in range(2)
                       for dt in range(DT) for s in range(2)]
            P_units += [(b, dt, sc) for b in range(1, BPC - 1) for dt in range(DT)
                        for sc in range(SC)]
            if os.environ.get("K_LASTPM", "1") == "1":
                # last batch sp-major: all heads' sp0 remaps land after its
                # first 12 P-units, so its M-halves interleave with its own
                # P phase instead of piling into the engine-bound tail
                P_units += [(BPC - 1, dt, 2 * sp + s) for sp in range(2)
                            for dt in range(DT) for s in range(2)]
            else:
                P_units += [(BPC - 1, dt, sc) for dt in range(DT)
                            for sc in range(SC)]
            M_units = [(0, h, sp) for sp in range(2) for h in range(NH)]
            M_units += [(b, h, sp) for b in range(1, BPC) for h in range(NH)
                        for sp in range(SH)]
            flat_state = {}
            small_state = {}
            remapped = set()
            xt_tiles = {0: (xt8_t0, xtb_t0)}
            p_i = m_i = 0

            def emit_P():
                nonlocal p_i
                b, dt, sc = P_units[p_i]
                # prefetch b+1: two single-descriptor whole-batch DMAs,
                # spread across dt 2 and 4 to avoid wire bursts
                if dt in (2, 4) and sc == 0 and b + 1 < BPC:
                    if dt == 2:
                        xt_tiles[b + 1] = (
                            xt_pool.tile([128, KF8, S], f8, tag="xt8", name="nx8"),
                            xt_pool.tile([128, KBF, S], bf16, tag="xtb", name="nxb"),
                        )
                    nx8, nxb = xt_tiles[b + 1]
                    if dt == 2:
                        nc.sync.dma_start(out=nx8, in_=xt8[b + 1])
                    else:
                        nc.sync.dma_start(out=nxb, in_=xtb[b + 1])
                if sc == 0:
                    flat_state[(b, dt)] = flat_pool.tile([128, S], bf16, tag="flat", name="flat_t")
                x8t, xbt = xt_tiles[b]
                proj_quarter(b, x8t, xbt, dt, sc, flat_state[(b, dt)])
                if sc % 2 == 1:
                    emit_remap_dtile(b, dt, sc // 2, flat_state[(b, dt)])
                    remapped.add((b, dt, sc // 2))
                if sc == SC - 1:
                    flat_state.pop((b, dt))
                p_i += 1

            def emit_M():
                nonlocal m_i
                b, h, sp = M_units[m_i]
                if (b, h) not in small_state:
                    small_state[(b, h)] = small_pool.tile(
                        [DH, SC], f32, tag="zp", name="zp", bufs=12
                    )
                e_t = e_pool.tile([DH, SHW], bf16, tag="e_t", name="e_t")
                zp = small_state[(b, h)]
                mlp_half(b, h, sp, e_t, zp, split_evac=(p_i >= SPLITP))
                if sp == SH - 1:
                    mlp_finish(b, h)
                    del small_state[(b, h)]
                m_i += 1

            def m_ready():
                if m_i >= len(M_units):
                    return False
                b, h, sp = M_units[m_i]
                return all((b, dt, sp) in remapped for dt in dts_of[h])

            while p_i < len(P_units) or m_i < len(M_units):
                lead = LEAD if p_i < len(P_units) - 16 else 0
                # pace M slightly faster than the 2:3 steady-state ratio so
                # the engine-bound M backlog is small when P runs out
                want_m = m_i * MRATD <= (p_i - lead) * MRAT
                if p_i < len(P_units) and not (want_m and m_ready()):
                    emit_P()
                elif m_ready():
                    emit_M()
                elif p_i < len(P_units):
                    emit_P()
                else:
                    # only unready M left: emit in order anyway (deps safe)
                    emit_M()

            # v = (sum of per-half accums) / Z + c, then transpose (96, 32)
            # -> (32, 96) and store.
            vq_sb = singles.tile([DH, BPC * NH], f32)
            nc.vector.tensor_reduce(
                out=vq_sb, in_=v_sb, axis=AX.X, op=OP.add
            )
            vn_sb = singles.tile([DH, BPC * NH], f32)
            nc.vector.tensor_mul(vn_sb, vq_sb, zr_sb)
            vc_sb = singles.tile([DH, BPC * NH], f32)
            nc.vector.tensor_add(vc_sb, vn_sb, cv_sb)
            vout_p = a2p_pool.tile([BPC * NH, DH], f32, tag="a2p")
            nc.tensor.matmul(vout_p, vc_sb, id_sb, start=True, stop=True)
            out_sb = singles.tile([BPC * NH, DH], f32)
            nc.scalar.copy(out=out_sb, in_=vout_p)
            nc.sync.dma_start(out=out[:], in_=out_sb)
    nc.compile()
    return nc


def get_nc():
    if "nc" not in _NC_CACHE:
        _NC_CACHE["nc"] = _build_nc()
    return _NC_CACHE["nc"]


def make_in_maps(token_embeddings, P_w, P_b, W1_w, W1_b, W2_w, W2_b):
    import ml_dtypes

    bf16 = ml_dtypes.bfloat16
    f8 = ml_dtypes.float8_e4m3
    TF8 = KF8 * 128  # features 0..511 in fp8

    X = np.asarray(token_embeddings, dtype=np.float32)
    # X^T per batch: (B, T, S) -> partition-major tiles [b, p, kt, s]
    XT = np.ascontiguousarray(X.transpose(0, 2, 1))
    XT8 = np.ascontiguousarray(
        XT[:, :TF8].astype(f8).reshape(B, KF8, 128, S).transpose(0, 2, 1, 3)
    )
    XTB = np.ascontiguousarray(
        XT[:, TF8:].astype(bf16).reshape(B, KBF, 128, S).transpose(0, 2, 1, 3)
    )

    P_cat = np.transpose(np.asarray(P_w, np.float32), (1, 0, 2)).reshape(T, NH * DH)
    P_s = P_cat * PSCALE
    p8_np = np.ascontiguousarray(
        P_s[:TF8].reshape(KF8, 128, NH * DH).transpose(1, 0, 2)
    ).astype(f8)
    pb16_np = np.ascontiguousarray(
        P_s[TF8:].reshape(KBF, 128, NH * DH).transpose(1, 0, 2)
    ).astype(bf16)

    # DC correction: c = mean_s(Hi_exact - Hi_quant), per (b, h*d).
    # Effective quantized P as used on device (scale folded back out):
    Pq = np.concatenate([
        p8_np.transpose(1, 0, 2).reshape(TF8, NH * DH).astype(np.float32),
        pb16_np.transpose(1, 0, 2).reshape(KBF * 128, NH * DH).astype(np.float32),
    ], axis=0) / PSCALE
    mu = X.mean(axis=1, dtype=np.float64)  # (B, T) exact means
    # means of the quantized X, reordered from [b, p, kt] to kt-major T index
    muq8 = XT8.astype(np.float32).mean(axis=3, dtype=np.float64)  # (B, 128, KF8)
    muqb = XTB.astype(np.float32).mean(axis=3, dtype=np.float64)  # (B, 128, KBF)
    muq = np.concatenate([
        muq8.transpose(0, 2, 1).reshape(B, TF8),
        muqb.transpose(0, 2, 1).reshape(B, KBF * 128),
    ], axis=1)  # (B, T)
    cfull = (mu @ P_cat.astype(np.float64)
             - muq @ Pq.astype(np.float64)).astype(np.float32)  # (B, NH*DH)

    w1 = np.ascontiguousarray(
        np.asarray(W1_w, np.float32).transpose(1, 0, 2)
    ).astype(bf16)
    b1l = np.ascontiguousarray(
        np.asarray(W1_b, np.float32).reshape(NH, KC, 128).transpose(2, 0, 1)
    ).astype(np.float32)

    w2 = np.ascontiguousarray(
        np.asarray(W2_w, np.float32).reshape(NH, KC, 128, DH).transpose(2, 0, 1, 3)
    ).astype(bf16)

    pb = np.ascontiguousarray(
        np.asarray(P_b, np.float32).reshape(NH * DH).reshape(KT, 128).T
    ).astype(np.float32)
    ident = np.eye(DH, dtype=np.float32)

    in_maps = []
    for c in range(NCORES):
        # cvec[d, b*NH+h] matches v_sb column layout
        cv = np.ascontiguousarray(
            cfull[c * BPC:(c + 1) * BPC].reshape(BPC * NH, DH).T
        ).astype(np.float32)
        in_maps.append({
            "xt8": np.ascontiguousarray(XT8[c * BPC:(c + 1) * BPC]),
            "xtb": np.ascontiguousarray(XTB[c * BPC:(c + 1) * BPC]),
            "p8": p8_np,
            "pb16": pb16_np,
            "w1": w1,
            "b1l": b1l,
            "w2": w2,
            "pb": pb,
            "cvec": cv,
            "ident": ident,
        })
    return in_maps


def _reference_host(token_embeddings, attention_mask, P_w, P_b, W1_w, W1_b, W2_w, W2_b):
    """Exact numpy fallback (only used if the mask is not all-ones)."""
    X = np.asarray(token_embeddings, np.float64)
    Hi = np.einsum("bst,htd->bhsd", X, np.asarray(P_w, np.float64))
    Hi += np.asarray(P_b, np.float64)[None, :, None, :]
    A = np.einsum("bhsd,hde->bhse", Hi, np.asarray(W1_w, np.float64))
    A += np.asarray(W1_b, np.float64)[None, :, None, :]
    A = np.maximum(A, 0.0)
    A = np.einsum("bhse,hed->bhsd", A, np.asarray(W2_w, np.float64))
    A += np.asarray(W2_b, np.float64)[None, :, None, :]
    with np.errstate(divide="ignore"):
        logm = np.log(np.asarray(attention_mask, np.float64))[:, None, :, None]
    A = A + logm
    A = A - A.max(axis=2, keepdims=True)
    E = np.exp(A)
    A = E / E.sum(axis=2, keepdims=True)
    v = (Hi * A).sum(axis=2)
    return v.reshape(v.shape[0], NH * DH).astype(np.float32)


def kernel(**inputs):
    mask = np.asarray(inputs["attention_mask"], np.float32)
    if not np.all(mask == 1.0):
        return _reference_host(**inputs)

    from concourse.bass_utils import run_bass_kernel_spmd

    nc = get_nc()
    in_maps = make_in_maps(
        inputs["token_embeddings"], inputs["P_w"], inputs["P_b"],
        inputs["W1_w"], inputs["W1_b"], inputs["W2_w"], inputs["W2_b"],
    )
    res = run_bass_kernel_spmd(nc, in_maps, core_ids=list(range(NCORES)))
    outs = [
        np.asarray(r["out"], np.float32).reshape(BPC, NH * DH)
        for r in res.results
    ]
    return np.concatenate(outs, axis=0)
